# revision 7
# baseline (speedup 1.0000x reference)
"""Trainium2 Bass kernel for KANPolyLayer:
    y[b,o] = sum_{i,p} x[b,i]^p * coeffs[o,i,p] + bias[o],  p = 0..4

Math: y = sum_{p=1..4} (x^p) @ C_p^T + biascol, with C_p = coeffs[:,:,p]
and biascol[o] = bias[o] + sum_i coeffs[o,i,0] folded on host (the p=0
plane is a constant column; folding it is 0.003% of the FLOPs).

Two-phase mixed-precision stream (the p1/p2 planes carry only ~3% of
the output variance, so fp8 there costs little accuracy but halves
their PE time via DoubleRow):

  Phase A (fp8 DoubleRow): y12 = p1 @ C1'^T + p2 @ C2'^T with
    C' = 256*C cast to fp8e4m3 on host, powers cast to fp8 on-chip.
    DoubleRow packs the (p1,p2) pair as one 256-deep contraction per
    matmul at 2 fp8 cols/cycle -> 64 matmuls instead of 128.
    Per-bank mid-evacuation folds the 1/256 descale and the bias in a
    single ACT op: stash = psA * (1/256) + biascol.
  Phase B (bf16): psB accumulates p3/p4; final DVE add of psB + stash.

Measured end-to-end rel err ~1.0e-2 (gate 2e-2; verified against an
offline simulation of the same rounding chain).

Schedule notes (from perfetto/NTFF analysis):
- DMA on the two HWDGE queues: SP = x k-planes + fp8 coeffs + even
  outputs; ACT = bf16 (p3,p4) coeffs + bias + odd outputs.  One
  descriptor per k-plane (DMA_DIRECT2D costs ~0.6us of engine-queue
  time each); k0 split fine-grained so the first matmul's operands
  land first.
- Engine balance: scalar = squares (bf16) + mid-evacs; vector = fp8
  casts (phase A feed) then p3/p4 muls (phase B feed) then final adds.
- PE warmup matmuls read broadcast const-APs (written by the framework
  preamble) so they start the moment the PE exits the preamble and the
  HAM clock-gate reaches 2.4 GHz as the real stream begins.
- 8 PSUM banks (4 o-tiles x 2 b-halves); each phase's last NTAIL
  k-planes are emitted bank-contiguous so banks finish staggered and
  evacuation overlaps the stream.

The kernel computes yT = [o, b]; host transposes.

Sharding (8 cores): 4 batch groups x 2 out-dim groups.
  core c -> (bg, og) = (c // 2, c % 2)
Each core computes a disjoint (512 x 1024) block of yT; host gathers.
"""

from contextlib import ExitStack

import ml_dtypes
import numpy as np

import concourse.bacc as bacc
import concourse.bass as bass
import concourse.mybir as mybir
import concourse.tile as tile
from concourse.bass_utils import run_bass_kernel_spmd

F32 = mybir.dt.float32
BF16 = mybir.dt.bfloat16
FP8 = mybir.dt.float8e4

B, I, O = 4096, 1024, 1024  # batch, in_dim, out_dim
BW, OW = 4, 2               # batch groups x out-dim groups (8 cores)
BS, OS = B // BW, O // OW   # per-core batch (1024) and out (512)
NK = I // 128               # contraction tiles (8)
NT = OS // 128              # o-tiles (4)
NH = BS // 512              # b-halves (2)
NTAIL = 2                   # trailing k-planes emitted bank-contiguous
WN = 7                      # PE warmup matmuls (HAM clock-gate)
CSC = 256.0                 # fp8 coeff scale (power of 2, exact)

_CACHE: dict = {}


def _build():
    nc = bacc.Bacc("TRN2", target_bir_lowering=False, debug=False, num_devices=8)

    # [k, i, b] bf16: per-k contiguous; the DMA'd plane IS the p=1 power
    xt = nc.dram_tensor("xt", [NK, 128, BS], BF16, kind="ExternalInput")
    # fp8 coeffs for phase A: [k, i, ot, p12, o] = 256*C{1,2} in fp8
    c8 = nc.dram_tensor("c8", [NK, 128, NT, 2, 128], FP8, kind="ExternalInput")
    # bf16 coeffs for phase B: [k, i, p34*o]
    cb = nc.dram_tensor("cb", [NK, 128, 2 * OS], BF16, kind="ExternalInput")
    # [i, ot]: bias[o] + colsum(C0)[o] as per-partition scalars
    biasc = nc.dram_tensor("biasc", [128, NT], F32, kind="ExternalInput")
    yt = nc.dram_tensor("yt", [OS, BS], F32, kind="ExternalOutput")  # [o, b]

    with tile.TileContext(nc) as tc, ExitStack() as ctx:
        cons = ctx.enter_context(tc.tile_pool(name="cons", bufs=1))
        c8pool = ctx.enter_context(tc.tile_pool(name="c8p", bufs=1))
        cbpool = ctx.enter_context(tc.tile_pool(name="cbp", bufs=1))
        xpool = ctx.enter_context(tc.tile_pool(name="xin", bufs=2))
        ppool = ctx.enter_context(tc.tile_pool(name="pow", bufs=1))
        spool = ctx.enter_context(tc.tile_pool(name="stash", bufs=1))
        opool = ctx.enter_context(tc.tile_pool(name="out", bufs=3))
        pspool = ctx.enter_context(
            tc.tile_pool(name="ps", bufs=8, space=bass.MemorySpace.PSUM)
        )

        # 8 accumulation banks: (o-tile, b-half)
        ps = {}
        for ot in range(NT):
            for h in range(NH):
                ps[(ot, h)] = pspool.tile(
                    [128, 512], F32, tag="ps", name=f"ps_{ot}_{h}"
                )

        # PE warmup on framework-preamble const tiles (no memset dep)
        wl = nc.const_aps.tensor(1.0, [128, 128], BF16)
        wr = nc.const_aps.tensor(1.0, [128, 512], BF16)
        for w in range(WN):
            nc.tensor.matmul(
                ps[(0, 0)], wl, wr, start=True, stop=True,
                skip_group_check=True,
            )

        biasc_sb = cons.tile([128, NT], F32)

        # --- input DMA issue helpers ---
        xks = {}
        c8s = {}
        cbs = {}

        def issue_xk(k):
            xk = xpool.tile([128, BS], BF16, tag="xk", name=f"xk_{k}")
            if k == 0:
                nc.sync.dma_start(xk[:, 0:512], xt[0, :, 0:512])
                nc.sync.dma_start(xk[:, 512:1024], xt[0, :, 512:1024])
            else:
                nc.sync.dma_start(xk[:], xt[k])
            xks[k] = xk

        def issue_c8(k):
            c8t = c8pool.tile([128, NT, 2, 128], FP8, tag=f"c8_{k}",
                              name=f"c8_{k}")
            if k == 0:
                nc.sync.dma_start(c8t[:, 0], c8[0, :, 0])
                nc.sync.dma_start(c8t[:, 1:NT], c8[0, :, 1:NT])
            else:
                nc.sync.dma_start(c8t[:], c8[k])
            c8s[k] = c8t

        def issue_cb(k):
            cbt = cbpool.tile([128, 2 * OS], BF16, tag=f"cb_{k}",
                              name=f"cb_{k}")
            nc.scalar.dma_start(cbt[:], cb[k])
            cbs[k] = cbt

        issue_xk(0)
        issue_c8(0)
        issue_xk(1)
        issue_c8(1)
        issue_cb(0)
        issue_cb(1)

        # ---------------- phase A: fp8 DoubleRow (p1, p2) ----------------
        pw12 = {}   # (k, h) -> [128, 2, 512] fp8
        p2bs = {}   # (k, h) -> [128, 512] bf16
        for k in range(NK):
            xk = xks[k]
            for h in range(NH):
                sl = xk[:, h * 512:(h + 1) * 512]
                p2b = ppool.tile([128, 512], BF16, tag=f"p2_{k}_{h}",
                                 name=f"p2_{k}_{h}")
                nc.scalar.square(p2b[:], sl)
                pw = ppool.tile([128, 2, 512], FP8, tag=f"pw_{k}_{h}",
                                name=f"pw_{k}_{h}")
                nc.vector.tensor_copy(pw[:, 0], sl)       # fp8 cast of x
                nc.vector.tensor_copy(pw[:, 1], p2b[:])   # fp8 cast of x^2
                pw12[(k, h)] = pw
                p2bs[(k, h)] = p2b

            if k + 2 < NK:
                issue_xk(k + 2)
            if k + 1 < NK:
                issue_c8(k + 1)
            if k + 2 < NK:
                issue_cb(k + 2)
            if k == 3:
                nc.scalar.dma_start(biasc_sb[:], biasc[:])

            if k < NK - NTAIL:
                for ot in range(NT):
                    for h in range(NH):
                        nc.tensor.matmul(
                            ps[(ot, h)],
                            c8s[k][:, ot],
                            pw12[(k, h)][:],
                            start=(k == 0),
                            stop=False,
                            perf_mode=mybir.MatmulPerfMode.DoubleRow,
                        )

        # phase-A tail, bank-contiguous + mid-evac (descale + bias, one ACT)
        stash = {}
        for ot in range(NT):
            for h in range(NH):
                for k in range(NK - NTAIL, NK):
                    nc.tensor.matmul(
                        ps[(ot, h)],
                        c8s[k][:, ot],
                        pw12[(k, h)][:],
                        start=False,
                        stop=(k == NK - 1),
                        perf_mode=mybir.MatmulPerfMode.DoubleRow,
                    )
                st = spool.tile([128, 512], F32, tag=f"st_{ot}_{h}",
                                name=f"st_{ot}_{h}")
                nc.scalar.activation(
                    st[:], ps[(ot, h)][:],
                    mybir.ActivationFunctionType.Identity,
                    bias=biasc_sb[:, ot:ot + 1],
                    scale=1.0 / CSC,
                )
                stash[(ot, h)] = st

        # ---------------- phase B: bf16 (p3, p4) ----------------
        pows = {}
        for k in range(NK):
            xk = xks[k]
            for h in range(NH):
                sl = xk[:, h * 512:(h + 1) * 512]
                p2b = p2bs[(k, h)]
                p3 = ppool.tile([128, 512], BF16, tag=f"p3_{k}_{h}",
                                name=f"p3_{k}_{h}")
                p4 = ppool.tile([128, 512], BF16, tag=f"p4_{k}_{h}",
                                name=f"p4_{k}_{h}")
                nc.vector.tensor_mul(p3[:], p2b[:], sl)
                nc.vector.tensor_mul(p4[:], p2b[:], p2b[:])
                pows[(3, k, h)] = p3
                pows[(4, k, h)] = p4

            if k < NK - NTAIL:
                for p in (3, 4):
                    for ot in range(NT):
                        for h in range(NH):
                            nc.tensor.matmul(
                                ps[(ot, h)],
                                cbs[k][:, (p - 3) * OS + ot * 128:
                                       (p - 3) * OS + (ot + 1) * 128],
                                pows[(p, k, h)][:],
                                start=(k == 0 and p == 3),
                                stop=False,
                            )

        # phase-B tail, bank-contiguous + final add + output DMA
        ngroups = NT * NH
        gi = 0
        for ot in range(NT):
            for h in range(NH):
                for k in range(NK - NTAIL, NK):
                    for p in (3, 4):
                        nc.tensor.matmul(
                            ps[(ot, h)],
                            cbs[k][:, (p - 3) * OS + ot * 128:
                                   (p - 3) * OS + (ot + 1) * 128],
                            pows[(p, k, h)][:],
                            start=False,
                            stop=(k == NK - 1 and p == 4),
                        )
                o_sb = opool.tile([128, 512], F32, tag="o_sb", name=f"o_{ot}_{h}")
                nc.vector.tensor_add(
                    o_sb[:], ps[(ot, h)][:], stash[(ot, h)][:]
                )
                gi += 1
                row = yt[ot * 128:(ot + 1) * 128, :]
                if gi <= ngroups - 2:
                    eng = nc.sync if gi % 2 else nc.scalar
                    eng.dma_start(
                        row[:, h * 512:(h + 1) * 512], o_sb[:]
                    )
                else:
                    # last two groups: split across both queues
                    nc.sync.dma_start(
                        row[:, h * 512:h * 512 + 256], o_sb[:, 0:256]
                    )
                    nc.scalar.dma_start(
                        row[:, h * 512 + 256:(h + 1) * 512], o_sb[:, 256:512]
                    )

    nc.compile()
    return nc


def _get_nc():
    if "nc" not in _CACHE:
        _CACHE["nc"] = _build()
    return _CACHE["nc"]


def _make_in_maps(x, coeffs, bias):
    x = np.asarray(x, dtype=np.float32)
    coeffs = np.asarray(coeffs, dtype=np.float32)
    bias = np.asarray(bias, dtype=np.float32)
    f8 = ml_dtypes.float8_e4m3

    # x slices: [1024b, 1024i] -> [1024i, 1024b] bf16 -> [8k, 128, 1024]
    xts = [
        np.ascontiguousarray(x[bg * BS:(bg + 1) * BS, :].T)
        .astype(ml_dtypes.bfloat16)
        .reshape(NK, 128, BS)
        for bg in range(BW)
    ]
    c8s, cbs, biascs = [], [], []
    for og in range(OW):
        C = coeffs[og * OS:(og + 1) * OS, :, :]  # [512o, 1024i, 5]
        # fp8 planes p1,p2 scaled: [1024i, 2p, 512o] -> [8k,128,4ot,2p,128o]
        c12 = np.ascontiguousarray(
            (C[:, :, 1:3] * CSC).transpose(1, 2, 0)
        ).astype(f8)  # [1024i, 2, 512o]
        c12 = c12.reshape(NK, 128, 2, NT, 128).transpose(0, 1, 3, 2, 4)
        c8s.append(np.ascontiguousarray(c12))
        # bf16 planes p3,p4: [1024i, 2p, 512o] -> [8k, 128, 2*512]
        c34 = np.ascontiguousarray(
            C[:, :, 3:5].transpose(1, 2, 0)
        ).astype(ml_dtypes.bfloat16).reshape(NK, 128, 2 * OS)
        cbs.append(c34)
        bc = (
            bias[0, og * OS:(og + 1) * OS] + C[:, :, 0].sum(axis=1)
        ).astype(np.float32)
        biascs.append(np.ascontiguousarray(bc.reshape(NT, 128).T))
    in_maps = []
    for c in range(BW * OW):
        bg, og = c // OW, c % OW
        in_maps.append(
            {"xt": xts[bg], "c8": c8s[og], "cb": cbs[og], "biasc": biascs[og]}
        )
    return in_maps


def _gather(results):
    y = np.empty((B, O), dtype=np.float32)
    for c, res in enumerate(results):
        bg, og = c // OW, c % OW
        y[bg * BS:(bg + 1) * BS, og * OS:(og + 1) * OS] = res["yt"].T
    return y


def run(x, coeffs, bias, trace=False, **trace_kwargs):
    nc = _get_nc()
    in_maps = _make_in_maps(x, coeffs, bias)
    br = run_bass_kernel_spmd(
        nc, in_maps, list(range(BW * OW)), trace=trace, **trace_kwargs
    )
    return _gather(br.results), br


def kernel(x, coeffs, bias):
    out, _ = run(x, coeffs, bias)
    return out
